# revision 8
# baseline (speedup 1.0000x reference)
"""Trainium2 Bass kernel for nn_CropRoi (3D RoI crop + adaptive max pool).

Contract: kernel(**inputs) takes FULL unsharded inputs
  f:         [B=2, C=128, Df=24, Hf=24, Wf=24] float32 feature map
  inputs:    [B, 1, D=96, H=96, W=96] float32 (only shape used)
  proposals: [N=64, 8] float32 (batch, score, center_zyx, side_zyx)
  scale:     scalar int
and returns the FULL output [N, C, 7, 7, 7] float32.

Strategy: shard proposals across the 8 NeuronCores grouped by batch so
each core loads its batch's feature map ONCE per execution (a single
large-descriptor HBM->SBUF DMA of the z/y union of its crops, chunked
over z so pooling overlaps the load). All 21 separable adaptive-max-pool
reductions per proposal then read straight from SBUF; proposals
alternate between the Vector (DVE) and GpSimd engines so both reduce in
parallel. Stage order per proposal minimizes elements read (reduce the
longest dim first). Outputs stream back on the scalar HWDGE queue.
"""

import sys

if "/opt/trn_rl_repo" not in sys.path:
    sys.path.insert(0, "/opt/trn_rl_repo")

import itertools

import numpy as np

S_OUT = 7
NEG32 = np.float32(np.finfo(np.float32).min)
N_CORES = 8
N_LOAD_CHUNKS = 3


# ----------------------------------------------------------------- host geometry
def _bins_1d(L):
    """Adaptive-pool windows for length L split into S_OUT bins.

    Returns (starts, widths) relative to the crop origin. Matches the
    reference's floor/ceil bin edges; for L <= 0 all windows are empty.
    """
    i = np.arange(S_OUT, dtype=np.int64)
    starts = (i * L) // S_OUT
    ends = -((-(i + 1) * L) // S_OUT)
    widths = np.maximum(ends - starts, 0)
    return starts.astype(int), widths.astype(int)


def build_geometry(f_shape, proposals, scale):
    """Mirror the reference's crop-bound computation exactly (float32 ops)."""
    B, C, Df, Hf, Wf = f_shape
    maxd = np.array([Df, Hf, Wf], np.int32)
    p = np.asarray(proposals, np.float32)
    center = p[:, 2:5].astype(np.float32)
    side = p[:, 5:8].astype(np.float32)
    c0f = center - side / np.float32(2.0)
    c1f = c0f + side
    sc = np.float32(scale)
    c0 = np.floor(c0f / sc).astype(np.int32)
    c1 = np.ceil(c1f / sc).astype(np.int32)
    c0 = np.maximum(c0, 0)
    c1 = np.minimum(c1, maxd[None, :])
    b = np.clip(p[:, 0].astype(np.int32), 0, B - 1)

    geoms = []
    for n in range(p.shape[0]):
        L = (c1[n] - c0[n]).astype(int)
        g = {
            "b": int(b[n]),
            "orig": [int(v) for v in c0[n]],
            "L": [int(v) for v in L],
            "empty": bool((L <= 0).any()),
        }
        g["bins"] = [_bins_1d(L[0]), _bins_1d(L[1]), _bins_1d(L[2])]
        geoms.append(g)
    return geoms


def _bin_w(w):
    # DVE cycles per output element: w=1 runs as a copy on ACT/GpSimd
    # (discounted), w=2 is one tensor_tensor max (2 reads/cycle), w>=3 is
    # one reduce_max reading w elems/output at 1/cycle.
    if w == 1:
        return 0.4
    if w == 2:
        return 1.0
    return float(w)


def _stage_perm(L, bins):
    """Pick the reduction order (perm of axes 0,1,2) minimizing engine cycles."""
    axw = [sum(_bin_w(int(w)) for w in b[1]) for b in bins]
    best, best_cost = None, None
    for perm in itertools.permutations(range(3)):
        rem = list(L)
        cost = 0.0
        for a in perm:
            others = 1
            for i in range(3):
                if i != a:
                    others *= rem[i]
            cost += axw[a] * others
            rem[a] = S_OUT
        if best_cost is None or cost < best_cost:
            best, best_cost = perm, cost
    return best, best_cost


def assign_cores(geoms):
    """Group proposals by batch and split them over the 8 cores so each
    core only touches one batch's feature map. Returns per-core
    (batch, [(orig_idx, geom), ...]) with counts balanced."""
    by_batch = {}
    for i, g in enumerate(geoms):
        by_batch.setdefault(g["b"], []).append((i, g))
    batches = sorted(by_batch)
    n_total = len(geoms)
    # proportional core counts, at least 1 per non-empty batch
    counts = {b: len(by_batch[b]) for b in batches}
    cores_for = {}
    remaining = N_CORES
    for j, b in enumerate(batches):
        if j == len(batches) - 1:
            cores_for[b] = remaining
        else:
            c = max(1, round(N_CORES * counts[b] / n_total))
            c = min(c, remaining - (len(batches) - 1 - j))
            cores_for[b] = c
            remaining -= c
    assignments = []
    for b in batches:
        items = by_batch[b]
        k = cores_for[b]
        per = -(-len(items) // k)
        for ci in range(k):
            chunk = items[ci * per:(ci + 1) * per]
            assignments.append((b, chunk))
    while len(assignments) < N_CORES:
        assignments.append((batches[0], []))
    return assignments[:N_CORES]


# ----------------------------------------------------------------- bass program
def build_core_program(f_shape, batch, items):
    """One specialized program for one core: len(items) proposals, all
    from `batch`. items: list of (orig_idx, geom)."""
    import concourse.bacc as bacc
    import concourse.tile as tile
    import concourse.mybir as mybir

    B, C, Df, Hf, Wf = f_shape
    P = max(len(items), 1)
    nc = bacc.Bacc("TRN2", target_bir_lowering=False, debug=False, num_devices=1)
    f_ap = nc.dram_tensor("f", [B, C, Df, Hf, Wf], mybir.dt.float32,
                          kind="ExternalInput").ap()
    o_ap = nc.dram_tensor("o", [P, C, S_OUT, S_OUT, S_OUT], mybir.dt.float32,
                          kind="ExternalOutput").ap()

    live = [(slot, g) for slot, (_idx, g) in enumerate(items) if not g["empty"]]
    # union of crop extents in z and y (x rows stay full for contiguity)
    if live:
        zmin = min(g["orig"][0] for _s, g in live)
        zmax = max(g["orig"][0] + g["L"][0] for _s, g in live)
        ymin = min(g["orig"][1] for _s, g in live)
        ymax = max(g["orig"][1] + g["L"][1] for _s, g in live)
    else:
        zmin, zmax, ymin, ymax = 0, 1, 0, 1
    Zu, Yu = zmax - zmin, ymax - ymin

    # sort by z-extent end so early proposals only need the first chunks
    live.sort(key=lambda sg: sg[1]["orig"][0] + sg[1]["L"][0])

    with tile.TileContext(nc) as tc:
        with tc.tile_pool(name="fpool", bufs=1) as fpool, \
             tc.tile_pool(name="pool", bufs=4) as pool, \
             tc.tile_pool(name="opool", bufs=4) as opool:
            fsb = fpool.tile([C, Zu, Yu, Wf], mybir.dt.float32, tag="fsb")
            nchunks = min(N_LOAD_CHUNKS, Zu)
            bounds = [round(Zu * t / nchunks) for t in range(nchunks + 1)]
            for t in range(nchunks):
                z0, z1 = bounds[t], bounds[t + 1]
                if z1 > z0:
                    nc.sync.dma_start(
                        out=fsb[:, z0:z1],
                        in_=f_ap[batch, :, zmin + z0:zmin + z1, ymin:ymax, :],
                    )

            # fill empty proposals (rare) with float32 min
            for slot, (_idx, g) in enumerate(items):
                if g["empty"]:
                    yx = opool.tile([C, S_OUT, S_OUT, S_OUT], mybir.dt.float32,
                                    tag="yx")
                    nc.vector.memset(yx[:], float(NEG32))
                    nc.scalar.dma_start(out=o_ap[slot], in_=yx[:])

            # Engine split: DVE does the max math (tensor_tensor for w=2,
            # reduce_max for w>=3); width-1 bins are plain copies, routed to
            # the otherwise-idle ACT and GpSimd engines (greedy ns balance).
            copy_load = [0.0, 0.0]  # ACT, GpSimd

            def emit_copy(out_ap, plane, n_elem):
                act_ns = copy_load[0] + (224 + n_elem) * 0.833
                gps_ns = copy_load[1] + n_elem * 0.86 + 150
                if act_ns <= gps_ns:
                    copy_load[0] = act_ns
                    nc.scalar.copy(out=out_ap, in_=plane)
                else:
                    copy_load[1] = gps_ns
                    nc.gpsimd.tensor_copy(out=out_ap, in_=plane)

            for slot, g in live:
                Lz, Ly, Lx = g["L"]
                perm, _cost = _stage_perm(g["L"], g["bins"])

                # crop offsets within fsb (x offset indexes full-width rows)
                off = [g["orig"][0] - zmin, g["orig"][1] - ymin, g["orig"][2]]
                dims = [Lz, Ly, Lx]
                src_tile, src_off = fsb, off
                names = ["z", "y", "x"]
                out_t = None
                for si, axis in enumerate(perm):
                    # dims[a] == -1 marks already-reduced axes (size S_OUT)
                    odims = [S_OUT if (a == axis or dims[a] == -1) else dims[a]
                             for a in range(3)]
                    shape = [C] + odims
                    if si == 2:
                        out_t = opool.tile([C, S_OUT, S_OUT, S_OUT],
                                           mybir.dt.float32, tag="yx")
                    else:
                        out_t = pool.tile(shape, mybir.dt.float32,
                                          tag=f"t{si}")
                    starts, widths = g["bins"][axis]
                    order = [names[a] for a in range(3) if a != axis] + [names[axis]]
                    pat = f"c z y x -> c {' '.join(order)}"
                    for i in range(S_OUT):
                        s0 = src_off[axis] + int(starts[i])
                        w = int(widths[i])
                        base = []
                        for a in range(3):
                            if a == axis:
                                base.append(None)
                            else:
                                d = S_OUT if dims[a] == -1 else dims[a]
                                base.append(slice(src_off[a], src_off[a] + d))
                        out_idx = tuple(i if a == axis else slice(None)
                                        for a in range(3))
                        out_ap = out_t[(slice(None), *out_idx)]
                        n_elem = 1
                        for a in range(3):
                            if a != axis:
                                n_elem *= S_OUT if dims[a] == -1 else dims[a]
                        if w == 1:
                            sl = [s0 if a == axis else base[a]
                                  for a in range(3)]
                            emit_copy(out_ap, src_tile[(slice(None), *sl)],
                                      n_elem)
                        elif w == 2:
                            p0 = src_tile[(slice(None), *[
                                s0 if a == axis else base[a] for a in range(3)])]
                            p1 = src_tile[(slice(None), *[
                                s0 + 1 if a == axis else base[a]
                                for a in range(3)])]
                            nc.vector.tensor_tensor(out=out_ap, in0=p0, in1=p1,
                                                    op=mybir.AluOpType.max)
                        else:
                            sl = [slice(s0, s0 + w) if a == axis else base[a]
                                  for a in range(3)]
                            src = src_tile[(slice(None), *sl)].rearrange(pat)
                            nc.vector.reduce_max(out=out_ap, in_=src,
                                                 axis=mybir.AxisListType.X)
                    dims[axis] = -1
                    src_tile, src_off = out_t, [0, 0, 0]
                nc.scalar.dma_start(out=o_ap[slot], in_=out_t[:])
    nc.compile()
    return nc


# ----------------------------------------------------------------- jax dispatch
def _make_jit(nc):
    """Mirror bass2jax.run_bass_via_pjrt's single-core body as a reusable jit."""
    import jax
    import concourse.mybir as mybir
    from concourse.bass2jax import (
        _bass_exec_p,
        install_neuronx_cc_hook,
        partition_id_tensor,
    )

    install_neuronx_cc_hook()

    partition_name = nc.partition_id_tensor.name if nc.partition_id_tensor else None
    in_names, out_names, out_avals, zero_outs = [], [], [], []
    for alloc in nc.m.functions[0].allocations:
        if not isinstance(alloc, mybir.MemoryLocationSet):
            continue
        name = alloc.memorylocations[0].name
        if alloc.kind == "ExternalInput":
            if name != partition_name:
                in_names.append(name)
        elif alloc.kind == "ExternalOutput":
            out_names.append(name)
            shape = tuple(alloc.tensor_shape)
            dtype = mybir.dt.np(alloc.dtype)
            out_avals.append(jax.core.ShapedArray(shape, dtype))
            zero_outs.append(np.zeros(shape, dtype))
    n_params = len(in_names)
    all_names = tuple(
        in_names + out_names + ([partition_name] if partition_name else [])
    )
    donate = tuple(range(n_params, n_params + len(out_names)))

    def _body(*args):
        operands = list(args)
        if partition_name is not None:
            operands.append(partition_id_tensor())
        outs = _bass_exec_p.bind(
            *operands,
            out_avals=tuple(out_avals),
            in_names=all_names,
            out_names=tuple(out_names),
            lowering_input_output_aliases=(),
            sim_require_finite=True,
            sim_require_nnan=True,
            nc=nc,
        )
        return tuple(outs)

    fn = jax.jit(_body, donate_argnums=donate, keep_unused=True)
    return fn, list(in_names), list(out_names), zero_outs


def _make_chain_jit(nc, reps):
    """Jit that runs the program `reps` times back-to-back on device,
    chaining the output buffer through, to measure per-execution HW time
    without per-rep host dispatch."""
    import jax
    import concourse.mybir as mybir
    from concourse.bass2jax import (
        _bass_exec_p,
        install_neuronx_cc_hook,
        partition_id_tensor,
    )

    install_neuronx_cc_hook()

    partition_name = nc.partition_id_tensor.name if nc.partition_id_tensor else None
    in_names, out_names, out_avals = [], [], []
    for alloc in nc.m.functions[0].allocations:
        if not isinstance(alloc, mybir.MemoryLocationSet):
            continue
        name = alloc.memorylocations[0].name
        if alloc.kind == "ExternalInput":
            if name != partition_name:
                in_names.append(name)
        elif alloc.kind == "ExternalOutput":
            out_names.append(name)
            out_avals.append(
                jax.core.ShapedArray(tuple(alloc.tensor_shape),
                                     mybir.dt.np(alloc.dtype))
            )
    all_names = tuple(
        in_names + out_names + ([partition_name] if partition_name else [])
    )

    def _step(_i, o, f):
        operands = [f, o]
        if partition_name is not None:
            operands.append(partition_id_tensor())
        (o,) = _bass_exec_p.bind(
            *operands,
            out_avals=tuple(out_avals),
            in_names=all_names,
            out_names=tuple(out_names),
            lowering_input_output_aliases=(),
            sim_require_finite=True,
            sim_require_nnan=True,
            nc=nc,
        )
        return o

    def _body(f, o):
        return jax.lax.fori_loop(0, reps, lambda i, o: _step(i, o, f), o)

    return jax.jit(_body, donate_argnums=(1,), keep_unused=True)


class CompiledKernel:
    """8 specialized per-core programs plus their jitted entry points."""

    def __init__(self, f_shape, geoms):
        import jax

        self.devices = jax.devices()[:N_CORES]
        assert len(self.devices) == N_CORES
        self.assignments = assign_cores(geoms)
        self.per_core = []
        self.ncs = []
        for k in range(N_CORES):
            batch, items = self.assignments[k]
            nc = build_core_program(f_shape, batch, items)
            self.ncs.append(nc)
            self.per_core.append(_make_jit(nc))

    def run(self, f):
        import jax

        outs = []
        for k, (fn, in_names, _out_names, zero_outs) in enumerate(self.per_core):
            assert in_names == ["f"]
            with jax.default_device(self.devices[k]):
                outs.append(fn(f, *[z.copy() for z in zero_outs]))
        return [np.asarray(o[0]) for o in outs]


def kernel(**inputs):
    f = np.ascontiguousarray(np.asarray(inputs["f"], dtype=np.float32))
    proposals = np.asarray(inputs["proposals"], dtype=np.float32)
    scale = int(np.asarray(inputs["scale"]))
    geoms = build_geometry(f.shape, proposals, scale)
    ck = CompiledKernel(f.shape, geoms)
    kernel.last_compiled = ck  # reused by test.py for benchmarking
    kernel.last_f = f
    parts = ck.run(f)
    N = proposals.shape[0]
    C = f.shape[1]
    out = np.empty((N, C, S_OUT, S_OUT, S_OUT), np.float32)
    for k in range(N_CORES):
        _batch, items = ck.assignments[k]
        for slot, (orig_idx, _g) in enumerate(items):
            out[orig_idx] = parts[k][slot]
    return out


kernel.last_compiled = None
kernel.last_f = None


# revision 9
# speedup vs baseline: 3.9921x; 3.9921x over previous
"""Trainium2 Bass kernel for nn_CropRoi (3D RoI crop + adaptive max pool).

Contract: kernel(**inputs) takes FULL unsharded inputs
  f:         [B=2, C=128, Df=24, Hf=24, Wf=24] float32 feature map
  inputs:    [B, 1, D=96, H=96, W=96] float32 (only shape used)
  proposals: [N=64, 8] float32 (batch, score, center_zyx, side_zyx)
  scale:     scalar int
and returns the FULL output [N, C, 7, 7, 7] float32.

Strategy: shard proposals across the 8 NeuronCores grouped by batch so
each core loads its batch's feature map ONCE per execution (a single
large-descriptor HBM->SBUF DMA of the z/y union of its crops, chunked
over z so pooling overlaps the load). All 21 separable adaptive-max-pool
reductions per proposal then read straight from SBUF; proposals
alternate between the Vector (DVE) and GpSimd engines so both reduce in
parallel. Stage order per proposal minimizes elements read (reduce the
longest dim first). Outputs stream back on the scalar HWDGE queue.
"""

import sys

if "/opt/trn_rl_repo" not in sys.path:
    sys.path.insert(0, "/opt/trn_rl_repo")

import itertools

import numpy as np

S_OUT = 7
NEG32 = np.float32(np.finfo(np.float32).min)
N_CORES = 8
N_LOAD_CHUNKS = 3


# ----------------------------------------------------------------- host geometry
def _bins_1d(L):
    """Adaptive-pool windows for length L split into S_OUT bins.

    Returns (starts, widths) relative to the crop origin. Matches the
    reference's floor/ceil bin edges; for L <= 0 all windows are empty.
    """
    i = np.arange(S_OUT, dtype=np.int64)
    starts = (i * L) // S_OUT
    ends = -((-(i + 1) * L) // S_OUT)
    widths = np.maximum(ends - starts, 0)
    return starts.astype(int), widths.astype(int)


def build_geometry(f_shape, proposals, scale):
    """Mirror the reference's crop-bound computation exactly (float32 ops)."""
    B, C, Df, Hf, Wf = f_shape
    maxd = np.array([Df, Hf, Wf], np.int32)
    p = np.asarray(proposals, np.float32)
    center = p[:, 2:5].astype(np.float32)
    side = p[:, 5:8].astype(np.float32)
    c0f = center - side / np.float32(2.0)
    c1f = c0f + side
    sc = np.float32(scale)
    c0 = np.floor(c0f / sc).astype(np.int32)
    c1 = np.ceil(c1f / sc).astype(np.int32)
    c0 = np.maximum(c0, 0)
    c1 = np.minimum(c1, maxd[None, :])
    b = np.clip(p[:, 0].astype(np.int32), 0, B - 1)

    geoms = []
    for n in range(p.shape[0]):
        L = (c1[n] - c0[n]).astype(int)
        g = {
            "b": int(b[n]),
            "orig": [int(v) for v in c0[n]],
            "L": [int(v) for v in L],
            "empty": bool((L <= 0).any()),
        }
        g["bins"] = [_bins_1d(L[0]), _bins_1d(L[1]), _bins_1d(L[2])]
        geoms.append(g)
    return geoms


def _bin_w(w):
    # DVE cycles per output element: w=1 runs as a copy on ACT/GpSimd
    # (discounted), w=2 is one tensor_tensor max (2 reads/cycle), w>=3 is
    # one reduce_max reading w elems/output at 1/cycle.
    if w == 1:
        return 0.4
    if w == 2:
        return 1.0
    return float(w)


def _stage_perm(L, bins):
    """Pick the reduction order (perm of axes 0,1,2) minimizing engine cycles."""
    axw = [sum(_bin_w(int(w)) for w in b[1]) for b in bins]
    best, best_cost = None, None
    for perm in itertools.permutations(range(3)):
        rem = list(L)
        cost = 0.0
        for a in perm:
            others = 1
            for i in range(3):
                if i != a:
                    others *= rem[i]
            cost += axw[a] * others
            rem[a] = S_OUT
        if best_cost is None or cost < best_cost:
            best, best_cost = perm, cost
    return best, best_cost


def assign_cores(geoms):
    """Group proposals by batch and split them over the 8 cores so each
    core only touches one batch's feature map. Returns per-core
    (batch, [(orig_idx, geom), ...]) with counts balanced."""
    by_batch = {}
    for i, g in enumerate(geoms):
        by_batch.setdefault(g["b"], []).append((i, g))
    batches = sorted(by_batch)
    n_total = len(geoms)
    # proportional core counts, at least 1 per non-empty batch
    counts = {b: len(by_batch[b]) for b in batches}
    cores_for = {}
    remaining = N_CORES
    for j, b in enumerate(batches):
        if j == len(batches) - 1:
            cores_for[b] = remaining
        else:
            c = max(1, round(N_CORES * counts[b] / n_total))
            c = min(c, remaining - (len(batches) - 1 - j))
            cores_for[b] = c
            remaining -= c
    assignments = []
    for b in batches:
        items = by_batch[b]
        k = cores_for[b]
        per = -(-len(items) // k)
        for ci in range(k):
            chunk = items[ci * per:(ci + 1) * per]
            assignments.append((b, chunk))
    while len(assignments) < N_CORES:
        assignments.append((batches[0], []))
    return assignments[:N_CORES]


# ----------------------------------------------------------------- bass program
def build_core_program(f_shape, batch, items):
    """One specialized program for one core: len(items) proposals, all
    from `batch`. items: list of (orig_idx, geom)."""
    import concourse.bacc as bacc
    import concourse.tile as tile
    import concourse.mybir as mybir

    B, C, Df, Hf, Wf = f_shape
    P = max(len(items), 1)
    nc = bacc.Bacc("TRN2", target_bir_lowering=False, debug=False, num_devices=1)
    f_ap = nc.dram_tensor("f", [B, C, Df, Hf, Wf], mybir.dt.float32,
                          kind="ExternalInput").ap()
    o_ap = nc.dram_tensor("o", [P, C, S_OUT, S_OUT, S_OUT], mybir.dt.float32,
                          kind="ExternalOutput").ap()

    live = [(slot, g) for slot, (_idx, g) in enumerate(items) if not g["empty"]]
    # union of crop extents in z and y (x rows stay full for contiguity)
    if live:
        zmin = min(g["orig"][0] for _s, g in live)
        zmax = max(g["orig"][0] + g["L"][0] for _s, g in live)
        ymin = min(g["orig"][1] for _s, g in live)
        ymax = max(g["orig"][1] + g["L"][1] for _s, g in live)
    else:
        zmin, zmax, ymin, ymax = 0, 1, 0, 1
    Zu, Yu = zmax - zmin, ymax - ymin

    # sort by z-extent end so early proposals only need the first chunks
    live.sort(key=lambda sg: sg[1]["orig"][0] + sg[1]["L"][0])

    with tile.TileContext(nc) as tc:
        with tc.tile_pool(name="fpool", bufs=1) as fpool, \
             tc.tile_pool(name="pool", bufs=4) as pool, \
             tc.tile_pool(name="opool", bufs=4) as opool:
            fsb = fpool.tile([C, Zu, Yu, Wf], mybir.dt.float32, tag="fsb")
            nchunks = min(N_LOAD_CHUNKS, Zu)
            bounds = [round(Zu * t / nchunks) for t in range(nchunks + 1)]
            for t in range(nchunks):
                z0, z1 = bounds[t], bounds[t + 1]
                if z1 > z0:
                    nc.sync.dma_start(
                        out=fsb[:, z0:z1],
                        in_=f_ap[batch, :, zmin + z0:zmin + z1, ymin:ymax, :],
                    )

            # fill empty proposals (rare) with float32 min
            for slot, (_idx, g) in enumerate(items):
                if g["empty"]:
                    yx = opool.tile([C, S_OUT, S_OUT, S_OUT], mybir.dt.float32,
                                    tag="yx")
                    nc.vector.memset(yx[:], float(NEG32))
                    nc.scalar.dma_start(out=o_ap[slot], in_=yx[:])

            # Engine split: DVE does the max math (tensor_tensor for w=2,
            # reduce_max for w>=3); width-1 bins are plain copies, routed to
            # the otherwise-idle ACT and GpSimd engines (greedy ns balance).
            copy_load = [0.0, 0.0]  # ACT, GpSimd

            def emit_copy(out_ap, plane, n_elem):
                act_ns = copy_load[0] + (224 + n_elem) * 0.833
                gps_ns = copy_load[1] + n_elem * 0.86 + 150
                if act_ns <= gps_ns:
                    copy_load[0] = act_ns
                    nc.scalar.copy(out=out_ap, in_=plane)
                else:
                    copy_load[1] = gps_ns
                    nc.gpsimd.tensor_copy(out=out_ap, in_=plane)

            for slot, g in live:
                Lz, Ly, Lx = g["L"]
                perm, _cost = _stage_perm(g["L"], g["bins"])

                # crop offsets within fsb (x offset indexes full-width rows)
                off = [g["orig"][0] - zmin, g["orig"][1] - ymin, g["orig"][2]]
                dims = [Lz, Ly, Lx]
                src_tile, src_off = fsb, off
                names = ["z", "y", "x"]
                out_t = None
                for si, axis in enumerate(perm):
                    # dims[a] == -1 marks already-reduced axes (size S_OUT)
                    odims = [S_OUT if (a == axis or dims[a] == -1) else dims[a]
                             for a in range(3)]
                    shape = [C] + odims
                    if si == 2:
                        out_t = opool.tile([C, S_OUT, S_OUT, S_OUT],
                                           mybir.dt.float32, tag="yx")
                    else:
                        out_t = pool.tile(shape, mybir.dt.float32,
                                          tag=f"t{si}")
                    starts, widths = g["bins"][axis]
                    order = [names[a] for a in range(3) if a != axis] + [names[axis]]
                    pat = f"c z y x -> c {' '.join(order)}"
                    for i in range(S_OUT):
                        s0 = src_off[axis] + int(starts[i])
                        w = int(widths[i])
                        base = []
                        for a in range(3):
                            if a == axis:
                                base.append(None)
                            else:
                                d = S_OUT if dims[a] == -1 else dims[a]
                                base.append(slice(src_off[a], src_off[a] + d))
                        out_idx = tuple(i if a == axis else slice(None)
                                        for a in range(3))
                        out_ap = out_t[(slice(None), *out_idx)]
                        n_elem = 1
                        for a in range(3):
                            if a != axis:
                                n_elem *= S_OUT if dims[a] == -1 else dims[a]
                        if w == 1:
                            sl = [s0 if a == axis else base[a]
                                  for a in range(3)]
                            emit_copy(out_ap, src_tile[(slice(None), *sl)],
                                      n_elem)
                        elif w == 2:
                            p0 = src_tile[(slice(None), *[
                                s0 if a == axis else base[a] for a in range(3)])]
                            p1 = src_tile[(slice(None), *[
                                s0 + 1 if a == axis else base[a]
                                for a in range(3)])]
                            nc.vector.tensor_tensor(out=out_ap, in0=p0, in1=p1,
                                                    op=mybir.AluOpType.max)
                        else:
                            sl = [slice(s0, s0 + w) if a == axis else base[a]
                                  for a in range(3)]
                            src = src_tile[(slice(None), *sl)].rearrange(pat)
                            nc.vector.reduce_max(out=out_ap, in_=src,
                                                 axis=mybir.AxisListType.X)
                    dims[axis] = -1
                    src_tile, src_off = out_t, [0, 0, 0]
                nc.scalar.dma_start(out=o_ap[slot], in_=out_t[:])
    nc.compile()
    return nc


# ----------------------------------------------------------------- jax dispatch
def _make_jit(nc):
    """Mirror bass2jax.run_bass_via_pjrt's single-core body as a reusable jit."""
    import jax
    import concourse.mybir as mybir
    from concourse.bass2jax import (
        _bass_exec_p,
        install_neuronx_cc_hook,
        partition_id_tensor,
    )

    install_neuronx_cc_hook()

    partition_name = nc.partition_id_tensor.name if nc.partition_id_tensor else None
    in_names, out_names, out_avals, zero_outs = [], [], [], []
    for alloc in nc.m.functions[0].allocations:
        if not isinstance(alloc, mybir.MemoryLocationSet):
            continue
        name = alloc.memorylocations[0].name
        if alloc.kind == "ExternalInput":
            if name != partition_name:
                in_names.append(name)
        elif alloc.kind == "ExternalOutput":
            out_names.append(name)
            shape = tuple(alloc.tensor_shape)
            dtype = mybir.dt.np(alloc.dtype)
            out_avals.append(jax.core.ShapedArray(shape, dtype))
            zero_outs.append(np.zeros(shape, dtype))
    n_params = len(in_names)
    all_names = tuple(
        in_names + out_names + ([partition_name] if partition_name else [])
    )
    donate = tuple(range(n_params, n_params + len(out_names)))

    def _body(*args):
        operands = list(args)
        if partition_name is not None:
            operands.append(partition_id_tensor())
        outs = _bass_exec_p.bind(
            *operands,
            out_avals=tuple(out_avals),
            in_names=all_names,
            out_names=tuple(out_names),
            lowering_input_output_aliases=(),
            sim_require_finite=True,
            sim_require_nnan=True,
            nc=nc,
        )
        return tuple(outs)

    fn = jax.jit(_body, donate_argnums=donate, keep_unused=True)
    return fn, list(in_names), list(out_names), zero_outs


def _make_chain_jit(nc, reps):
    """Jit that runs the program `reps` times back-to-back on device,
    chaining the output buffer through, to measure per-execution HW time
    without per-rep host dispatch."""
    import jax
    import concourse.mybir as mybir
    from concourse.bass2jax import (
        _bass_exec_p,
        install_neuronx_cc_hook,
        partition_id_tensor,
    )

    install_neuronx_cc_hook()

    partition_name = nc.partition_id_tensor.name if nc.partition_id_tensor else None
    in_names, out_names, out_avals = [], [], []
    for alloc in nc.m.functions[0].allocations:
        if not isinstance(alloc, mybir.MemoryLocationSet):
            continue
        name = alloc.memorylocations[0].name
        if alloc.kind == "ExternalInput":
            if name != partition_name:
                in_names.append(name)
        elif alloc.kind == "ExternalOutput":
            out_names.append(name)
            out_avals.append(
                jax.core.ShapedArray(tuple(alloc.tensor_shape),
                                     mybir.dt.np(alloc.dtype))
            )
    all_names = tuple(
        in_names + out_names + ([partition_name] if partition_name else [])
    )

    def _step(o, f):
        operands = [f, o]
        if partition_name is not None:
            operands.append(partition_id_tensor())
        (o,) = _bass_exec_p.bind(
            *operands,
            out_avals=tuple(out_avals),
            in_names=all_names,
            out_names=tuple(out_names),
            lowering_input_output_aliases=(),
            sim_require_finite=True,
            sim_require_nnan=True,
            nc=nc,
        )
        return o

    def _body(f, o):
        for _ in range(reps):
            o = _step(o, f)
        return o

    return jax.jit(_body, donate_argnums=(1,), keep_unused=True)


class CompiledKernel:
    """8 specialized per-core programs plus their jitted entry points."""

    def __init__(self, f_shape, geoms):
        import jax

        self.devices = jax.devices()[:N_CORES]
        assert len(self.devices) == N_CORES
        self.assignments = assign_cores(geoms)
        self.per_core = []
        self.ncs = []
        for k in range(N_CORES):
            batch, items = self.assignments[k]
            nc = build_core_program(f_shape, batch, items)
            self.ncs.append(nc)
            self.per_core.append(_make_jit(nc))

    def run(self, f):
        import jax

        outs = []
        for k, (fn, in_names, _out_names, zero_outs) in enumerate(self.per_core):
            assert in_names == ["f"]
            with jax.default_device(self.devices[k]):
                outs.append(fn(f, *[z.copy() for z in zero_outs]))
        return [np.asarray(o[0]) for o in outs]


def kernel(**inputs):
    f = np.ascontiguousarray(np.asarray(inputs["f"], dtype=np.float32))
    proposals = np.asarray(inputs["proposals"], dtype=np.float32)
    scale = int(np.asarray(inputs["scale"]))
    geoms = build_geometry(f.shape, proposals, scale)
    ck = CompiledKernel(f.shape, geoms)
    kernel.last_compiled = ck  # reused by test.py for benchmarking
    kernel.last_f = f
    parts = ck.run(f)
    N = proposals.shape[0]
    C = f.shape[1]
    out = np.empty((N, C, S_OUT, S_OUT, S_OUT), np.float32)
    for k in range(N_CORES):
        _batch, items = ck.assignments[k]
        for slot, (orig_idx, _g) in enumerate(items):
            out[orig_idx] = parts[k][slot]
    return out


kernel.last_compiled = None
kernel.last_f = None
